# revision 8
# baseline (speedup 1.0000x reference)
"""Longformer local self-attention on 8 Trainium2 NeuronCores.

Sharding: sequence-parallel. 32 (batch, 256-query-chunk) pairs total;
each core takes one batch element's 1024-query slice (4 chunks) plus a
256-position K/V halo on each side, recomputing K/V projections in the
halo so no inter-core communication is needed.

Device algorithm per core (all heads):
  - QKV projections as matmuls on a host-pretransposed x^T with an
    appended ones-row that folds the biases (and the V "ones column"
    used for the softmax denominator) into the weight matrix.
  - Per (chunk, head): scores^T[y,x] = K_win^T Q_chunk in PSUM, then
    exp(scores + key_bias) on ScalarE (key_bias holds the attention
    mask and -1e30 for out-of-range halo keys, as a per-partition
    activation bias), band mask via a 0/1 multiply, and the AV matmul
    with probs^T as the stationary operand. The extra V column of ones
    yields the softmax denominator in the same accumulation; a
    reciprocal + per-partition scalar multiply normalizes.
"""

import math

import numpy as np
import ml_dtypes

import concourse.bass as bass
import concourse.bacc as bacc
import concourse.tile as tile
from concourse import mybir
from concourse.bass_utils import run_bass_kernel_spmd

B, S, E = 2, 4096, 768
H, D = 12, 64
W = 256                      # one-sided window; chunk size
NCORES = 8
CHUNKS_PER_CORE = 4
SEQ_Q = CHUNKS_PER_CORE * W          # 1024 queries per core
SEQ_HALO = SEQ_Q + 2 * W             # 1536 K/V positions per core
KIN = E + 8                          # 768 + bias row (768) + 7 zero pad rows
NKT = 7                              # contraction tiles: 6x128 + 1x8
WCOLS = E + E + H * (D + 1)          # Wq^T | Wk^T | per-head [Wv^T | ones]
VCOLS = H * (D + 1)                  # 780
P = 128
NEG = -1e30

F32 = mybir.dt.float32
BF16 = mybir.dt.bfloat16

_cached = {}


def _ktile(k):
    """(row offset, row count) of contraction tile k."""
    return (k * P, P if k < 6 else KIN - 6 * P)


def _build_program(dt_mm):
    nc = bacc.Bacc("TRN2", target_bir_lowering=False)
    xT_d = nc.dram_tensor("xT", [KIN, SEQ_HALO], dt_mm, kind="ExternalInput")
    W_d = nc.dram_tensor("Wcat", [KIN, WCOLS], dt_mm, kind="ExternalInput")
    fm_d = nc.dram_tensor("fm", [P, SEQ_HALO // P], F32, kind="ExternalInput")
    band_d = nc.dram_tensor("bandT", [3 * W, W], dt_mm, kind="ExternalInput")
    out_d = nc.dram_tensor("out", [SEQ_Q, E], F32, kind="ExternalOutput")

    with tile.TileContext(nc) as tc:
        with (
            tc.tile_pool(name="singles", bufs=1) as singles,
            tc.tile_pool(name="ptp", bufs=12) as ptp,
            tc.tile_pool(name="osbp", bufs=4) as osbp,
            tc.tile_pool(name="recp", bufs=8) as recp,
            tc.tile_pool(name="pproj", bufs=2, space="PSUM") as pproj,
            tc.tile_pool(name="psc", bufs=4, space="PSUM") as psc,
            tc.tile_pool(name="psx", bufs=2, space="PSUM") as psx,
        ):
            # ---- load everything resident ----
            xts = []
            wts = []
            for k in range(NKT):
                ro, rc = _ktile(k)
                xt = singles.tile([rc, SEQ_HALO], dt_mm, name=f"xt{k}")
                nc.sync.dma_start(out=xt, in_=xT_d[ro:ro + rc, :])
                xts.append(xt)
                wt = singles.tile([rc, WCOLS], dt_mm, name=f"wt{k}")
                nc.sync.dma_start(out=wt, in_=W_d[ro:ro + rc, :])
                wts.append(wt)
            fmsb = singles.tile([P, SEQ_HALO // P], F32, name="fmsb")
            nc.sync.dma_start(out=fmsb, in_=fm_d[:, :])
            bandsb = []
            for t in range(6):
                bs = singles.tile([P, W], dt_mm, name=f"band{t}")
                nc.sync.dma_start(out=bs, in_=band_d[t * P:(t + 1) * P, :])
                bandsb.append(bs)

            # ---- projections ----
            # Q^T/K^T: [E_out(part) x seq(free)], Q over center 1024 cols.
            qt = [singles.tile([P, SEQ_Q], dt_mm, name=f"qt{m}") for m in range(6)]
            kt = [singles.tile([P, SEQ_HALO], dt_mm, name=f"kt{m}") for m in range(6)]
            for m in range(6):
                for dst, wofs, ncols, colofs in (
                    (qt[m], 0, SEQ_Q, W),
                    (kt[m], E, SEQ_HALO, 0),
                ):
                    for ns in range(ncols // 512):
                        ps = pproj.tile([P, 512], F32, tag="pp")
                        for k in range(NKT):
                            nc.tensor.matmul(
                                ps,
                                lhsT=wts[k][:, wofs + m * P: wofs + (m + 1) * P],
                                rhs=xts[k][:, colofs + ns * 512: colofs + (ns + 1) * 512],
                                start=(k == 0), stop=(k == NKT - 1),
                            )
                        nc.vector.tensor_copy(dst[:, ns * 512:(ns + 1) * 512], ps)
            # V_ext: [seq(part) x 780(free)], per-head 65 cols (64 + ones).
            vsb = [singles.tile([P, VCOLS], dt_mm, name=f"v{s}") for s in range(12)]
            for s in range(12):
                for nofs, nsz in ((0, 512), (512, VCOLS - 512)):
                    ps = pproj.tile([P, 512], F32, tag="pp")
                    for k in range(NKT):
                        nc.tensor.matmul(
                            ps[:, :nsz],
                            lhsT=xts[k][:, s * P:(s + 1) * P],
                            rhs=wts[k][:, 2 * E + nofs: 2 * E + nofs + nsz],
                            start=(k == 0), stop=(k == NKT - 1),
                        )
                    nc.vector.tensor_copy(vsb[s][:, nofs:nofs + nsz], ps[:, :nsz])

            # ---- attention ----
            for c in range(CHUNKS_PER_CORE):
                osb = [osbp.tile([P, E], F32, tag="osb", name=f"osb{c}_{i}")
                       for i in range(2)]
                for h in range(H):
                    mt, ho = h // 2, (h % 2) * 64
                    pts = []
                    for yt in range(6):
                        ps = psc.tile([P, W], F32, tag="sc")
                        nc.tensor.matmul(
                            ps,
                            lhsT=kt[mt][ho:ho + 64,
                                        c * W + yt * P: c * W + (yt + 1) * P],
                            rhs=qt[mt][ho:ho + 64, c * W:(c + 1) * W],
                            start=True, stop=True,
                        )
                        pt = ptp.tile([P, W], dt_mm, tag="pt")
                        nc.scalar.activation(
                            pt, ps, mybir.ActivationFunctionType.Exp,
                            bias=fmsb[:, c * 2 + yt: c * 2 + yt + 1],
                        )
                        if yt not in (2, 3):
                            nc.vector.tensor_mul(pt, pt, bandsb[yt])
                        pts.append(pt)
                    for xt_i in range(2):
                        cps = psx.tile([P, D + 1], F32, tag="cx")
                        for yt in range(6):
                            nc.tensor.matmul(
                                cps,
                                lhsT=pts[yt][:, xt_i * P:(xt_i + 1) * P],
                                rhs=vsb[c * 2 + yt][:, h * (D + 1):(h + 1) * (D + 1)],
                                start=(yt == 0), stop=(yt == 5),
                            )
                        rec = recp.tile([P, 1], F32, tag="rec")
                        nc.vector.reciprocal(rec, cps[:, D:D + 1])
                        nc.vector.tensor_scalar_mul(
                            osb[xt_i][:, h * D:(h + 1) * D], cps[:, :D], rec)
                for xt_i in range(2):
                    nc.sync.dma_start(
                        out=out_d[c * W + xt_i * P: c * W + (xt_i + 1) * P, :],
                        in_=osb[xt_i])
    nc.finalize()
    return nc


def _prep_inputs(hidden_states, attention_mask, Wq, bq, Wk, bk, Wv, bv, np_mm):
    scale = 1.0 / math.sqrt(D)
    Wcat = np.zeros((KIN, WCOLS), np.float32)
    Wcat[:E, :E] = Wq.T * scale
    Wcat[E, :E] = bq * scale
    Wcat[:E, E:2 * E] = Wk.T
    Wcat[E, E:2 * E] = bk
    for h in range(H):
        c0 = 2 * E + h * (D + 1)
        Wcat[:E, c0:c0 + D] = Wv.T[:, h * D:(h + 1) * D]
        Wcat[E, c0:c0 + D] = bv[h * D:(h + 1) * D]
        Wcat[E, c0 + D] = 1.0
    Wcat = Wcat.astype(np_mm)

    band = (np.arange(3 * W)[:, None] - np.arange(W)[None, :])
    bandT = ((band >= 0) & (band <= 2 * W)).astype(np_mm)

    fm_full = np.where(attention_mask[:, 0, 0, :] != 0.0,
                       np.float32(-10000.0), np.float32(0.0))

    in_maps = []
    for core in range(NCORES):
        b, g = core // CHUNKS_PER_CORE, core % CHUNKS_PER_CORE
        qlo = g * SEQ_Q
        lo, hi = qlo - W, qlo + SEQ_Q + W
        clo, chi = max(lo, 0), min(hi, S)
        xT = np.zeros((KIN, SEQ_HALO), np.float32)
        xT[:E, clo - lo:chi - lo] = hidden_states[b, clo:chi, :].T
        xT[E, :] = 1.0
        fm = np.full((SEQ_HALO,), NEG, np.float32)
        fm[clo - lo:chi - lo] = fm_full[b, clo:chi]
        in_maps.append({
            "xT": np.ascontiguousarray(xT.astype(np_mm)),
            "Wcat": Wcat,
            "fm": np.ascontiguousarray(fm.reshape(SEQ_HALO // P, P).T),
            "bandT": bandT,
        })
    return in_maps


def _get_program(dt_mm):
    if dt_mm not in _cached:
        _cached[dt_mm] = _build_program(dt_mm)
    return _cached[dt_mm]


def run(hidden_states, attention_mask, Wq, bq, Wk, bk, Wv, bv,
        dt_mm=BF16, np_mm=ml_dtypes.bfloat16, **run_kwargs):
    nc = _get_program(dt_mm)
    in_maps = _prep_inputs(
        np.asarray(hidden_states, np.float32),
        np.asarray(attention_mask, np.float32),
        np.asarray(Wq, np.float32), np.asarray(bq, np.float32),
        np.asarray(Wk, np.float32), np.asarray(bk, np.float32),
        np.asarray(Wv, np.float32), np.asarray(bv, np.float32),
        np_mm)
    res = run_bass_kernel_spmd(nc, in_maps, list(range(NCORES)), **run_kwargs)
    out = np.empty((B, S, E), np.float32)
    for core in range(NCORES):
        b, g = core // CHUNKS_PER_CORE, core % CHUNKS_PER_CORE
        out[b, g * SEQ_Q:(g + 1) * SEQ_Q, :] = res.results[core]["out"]
    return out, res


def kernel(**inputs):
    out, _ = run(**inputs)
    return out


def bench(inputs, iters=20, dt_mm=BF16, np_mm=None):
    """Steady-state device timing: jitted shard_map over 8 cores, inputs
    device-resident, no donation (kernel writes every output element)."""
    import time
    import jax
    import ml_dtypes
    from jax.experimental.shard_map import shard_map
    from jax.sharding import Mesh, PartitionSpec
    from concourse import bass2jax, mybir as mb

    if np_mm is None:
        np_mm = ml_dtypes.bfloat16 if dt_mm == BF16 else np.float32
    nc = _get_program(dt_mm)
    in_maps = _prep_inputs(
        np.asarray(inputs["hidden_states"], np.float32),
        np.asarray(inputs["attention_mask"], np.float32),
        np.asarray(inputs["Wq"], np.float32), np.asarray(inputs["bq"], np.float32),
        np.asarray(inputs["Wk"], np.float32), np.asarray(inputs["bk"], np.float32),
        np.asarray(inputs["Wv"], np.float32), np.asarray(inputs["bv"], np.float32),
        np_mm)
    bass2jax.install_neuronx_cc_hook()

    pid_name = nc.partition_id_tensor.name if nc.partition_id_tensor else None
    in_names, out_names, out_avals = [], [], []
    for alloc in nc.m.functions[0].allocations:
        if not isinstance(alloc, mb.MemoryLocationSet):
            continue
        name = alloc.memorylocations[0].name
        if alloc.kind == "ExternalInput":
            if name != pid_name:
                in_names.append(name)
        elif alloc.kind == "ExternalOutput":
            out_names.append(name)
            out_avals.append(jax.core.ShapedArray(
                tuple(alloc.tensor_shape), mb.dt.np(alloc.dtype)))
    bind_names = list(in_names) + ([pid_name] if pid_name else [])

    def _body(*args):
        operands = list(args)
        if pid_name:
            operands.append(bass2jax.partition_id_tensor())
        return tuple(bass2jax._bass_exec_p.bind(
            *operands, out_avals=tuple(out_avals), in_names=tuple(bind_names),
            out_names=tuple(out_names), lowering_input_output_aliases=(),
            sim_require_finite=True, sim_require_nnan=True, nc=nc))

    devices = jax.devices()[:NCORES]
    mesh = Mesh(np.asarray(devices), ("core",))
    fn = jax.jit(shard_map(
        _body, mesh=mesh,
        in_specs=(PartitionSpec("core"),) * len(in_names),
        out_specs=(PartitionSpec("core"),) * len(out_names),
        check_rep=False), keep_unused=True)
    from jax.sharding import NamedSharding
    shard = NamedSharding(mesh, PartitionSpec("core"))
    concat_in = [
        jax.device_put(
            np.concatenate([in_maps[c][n] for c in range(NCORES)], 0), shard)
        for n in in_names]
    r = fn(*concat_in)
    jax.block_until_ready(r)
    times = []
    for _ in range(iters):
        t0 = time.perf_counter()
        r = fn(*concat_in)
        jax.block_until_ready(r)
        times.append(time.perf_counter() - t0)
    out = np.asarray(r[out_names.index("out")]).reshape(NCORES, SEQ_Q, E)
    full = np.empty((B, S, E), np.float32)
    for core in range(NCORES):
        b, g = core // CHUNKS_PER_CORE, core % CHUNKS_PER_CORE
        full[b, g * SEQ_Q:(g + 1) * SEQ_Q, :] = out[core]
    return full, times


# revision 13
# speedup vs baseline: 540.5576x; 540.5576x over previous
"""Longformer local self-attention on 8 Trainium2 NeuronCores.

Sharding: sequence-parallel. 32 (batch, 256-query-chunk) pairs total;
each core takes one batch element's 1024-query slice (4 chunks) plus a
256-position K/V halo on each side, recomputing K/V projections in the
halo so no inter-core communication is needed.

Device algorithm per core (all heads):
  - QKV projections as matmuls on a host-pretransposed x^T with an
    appended ones-row that folds the biases (and the V "ones column"
    used for the softmax denominator) into the weight matrix.
  - Per (chunk, head): scores^T[y,x] = K_win^T Q_chunk in PSUM, then
    exp(scores + key_bias) on ScalarE (key_bias holds the attention
    mask and -1e30 for out-of-range halo keys, as a per-partition
    activation bias), band mask via a 0/1 multiply, and the AV matmul
    with probs^T as the stationary operand. The extra V column of ones
    yields the softmax denominator in the same accumulation; a
    reciprocal + per-partition scalar multiply normalizes.
"""

import math

import numpy as np
import ml_dtypes

import concourse.bass as bass
import concourse.bacc as bacc
import concourse.tile as tile
from concourse import mybir
from concourse.bass_utils import run_bass_kernel_spmd

B, S, E = 2, 4096, 768
H, D = 12, 64
W = 256                      # one-sided window; chunk size
NCORES = 8
CHUNKS_PER_CORE = 4
SEQ_Q = CHUNKS_PER_CORE * W          # 1024 queries per core
SEQ_HALO = SEQ_Q + 2 * W             # 1536 K/V positions per core
KIN = E + 8                          # 768 + bias row (768) + 7 zero pad rows
NKT = 7                              # contraction tiles: 6x128 + 1x8
WCOLS = E + E + H * (D + 1)          # Wq^T | Wk^T | per-head [Wv^T | ones]
VCOLS = H * (D + 1)                  # 780
P = 128
NEG = -1e30

F32 = mybir.dt.float32
BF16 = mybir.dt.bfloat16

_cached = {}


def _ktile(k):
    """(row offset, row count) of contraction tile k."""
    return (k * P, P if k < 6 else KIN - 6 * P)


def _loop_ctx(tc, reps):
    """Identity context for reps=1, else a Tile For_i timing loop."""
    import contextlib
    if reps == 1:
        return contextlib.nullcontext()
    et = mybir.EngineType
    return tc.For_i(0, reps, 1,
                    hint_engines=(et.PE, et.Activation, et.DVE, et.SP, et.Pool))


def _build_program(dt_mm, reps=1):
    nc = bacc.Bacc("TRN2", target_bir_lowering=False)
    xT_d = nc.dram_tensor("xT", [KIN, SEQ_HALO], dt_mm, kind="ExternalInput")
    W_d = nc.dram_tensor("Wcat", [KIN, WCOLS], dt_mm, kind="ExternalInput")
    fm_d = nc.dram_tensor("fm", [P, SEQ_HALO // P], F32, kind="ExternalInput")
    band_d = nc.dram_tensor("bandT", [3 * W, W], dt_mm, kind="ExternalInput")
    out_d = nc.dram_tensor("out", [SEQ_Q, E], F32, kind="ExternalOutput")

    with tile.TileContext(nc) as tc:
        with (
            tc.tile_pool(name="singles", bufs=1) as singles,
            tc.tile_pool(name="ptp", bufs=12) as ptp,
            tc.tile_pool(name="osbp", bufs=4) as osbp,
            tc.tile_pool(name="recp", bufs=8) as recp,
            tc.tile_pool(name="pproj", bufs=2, space="PSUM") as pproj,
            tc.tile_pool(name="psc", bufs=4, space="PSUM") as psc,
            tc.tile_pool(name="psx", bufs=2, space="PSUM") as psx,
            _loop_ctx(tc, reps),
        ):
            # ---- load everything resident ----
            xts = []
            wts = []
            for k in range(NKT):
                ro, rc = _ktile(k)
                xt = singles.tile([rc, SEQ_HALO], dt_mm, name=f"xt{k}")
                nc.sync.dma_start(out=xt, in_=xT_d[ro:ro + rc, :])
                xts.append(xt)
                wt = singles.tile([rc, WCOLS], dt_mm, name=f"wt{k}")
                nc.sync.dma_start(out=wt, in_=W_d[ro:ro + rc, :])
                wts.append(wt)
            fmsb = singles.tile([P, SEQ_HALO // P], F32, name="fmsb")
            nc.sync.dma_start(out=fmsb, in_=fm_d[:, :])
            bandsb = []
            for t in range(6):
                bs = singles.tile([P, W], dt_mm, name=f"band{t}")
                nc.sync.dma_start(out=bs, in_=band_d[t * P:(t + 1) * P, :])
                bandsb.append(bs)

            # ---- projections ----
            # Q^T/K^T: [E_out(part) x seq(free)], Q over center 1024 cols.
            qt = [singles.tile([P, SEQ_Q], dt_mm, name=f"qt{m}") for m in range(6)]
            kt = [singles.tile([P, SEQ_HALO], dt_mm, name=f"kt{m}") for m in range(6)]
            for m in range(6):
                for dst, wofs, ncols, colofs in (
                    (qt[m], 0, SEQ_Q, W),
                    (kt[m], E, SEQ_HALO, 0),
                ):
                    for ns in range(ncols // 512):
                        ps = pproj.tile([P, 512], F32, tag="pp")
                        for k in range(NKT):
                            nc.tensor.matmul(
                                ps,
                                lhsT=wts[k][:, wofs + m * P: wofs + (m + 1) * P],
                                rhs=xts[k][:, colofs + ns * 512: colofs + (ns + 1) * 512],
                                start=(k == 0), stop=(k == NKT - 1),
                            )
                        nc.vector.tensor_copy(dst[:, ns * 512:(ns + 1) * 512], ps)
            # V_ext: [seq(part) x 780(free)], per-head 65 cols (64 + ones).
            vsb = [singles.tile([P, VCOLS], dt_mm, name=f"v{s}") for s in range(12)]
            for s in range(12):
                for nofs, nsz in ((0, 512), (512, VCOLS - 512)):
                    ps = pproj.tile([P, 512], F32, tag="pp")
                    for k in range(NKT):
                        nc.tensor.matmul(
                            ps[:, :nsz],
                            lhsT=xts[k][:, s * P:(s + 1) * P],
                            rhs=wts[k][:, 2 * E + nofs: 2 * E + nofs + nsz],
                            start=(k == 0), stop=(k == NKT - 1),
                        )
                    nc.vector.tensor_copy(vsb[s][:, nofs:nofs + nsz], ps[:, :nsz])

            # ---- attention ----
            for c in range(CHUNKS_PER_CORE):
                osb = [osbp.tile([P, E], F32, tag="osb", name=f"osb{c}_{i}")
                       for i in range(2)]
                for h in range(H):
                    mt, ho = h // 2, (h % 2) * 64
                    pts = []
                    for yt in range(6):
                        ps = psc.tile([P, W], F32, tag="sc")
                        nc.tensor.matmul(
                            ps,
                            lhsT=kt[mt][ho:ho + 64,
                                        c * W + yt * P: c * W + (yt + 1) * P],
                            rhs=qt[mt][ho:ho + 64, c * W:(c + 1) * W],
                            start=True, stop=True,
                        )
                        pt = ptp.tile([P, W], dt_mm, tag="pt")
                        nc.scalar.activation(
                            pt, ps, mybir.ActivationFunctionType.Exp,
                            bias=fmsb[:, c * 2 + yt: c * 2 + yt + 1],
                        )
                        if yt not in (2, 3):
                            nc.vector.tensor_mul(pt, pt, bandsb[yt])
                        pts.append(pt)
                    for xt_i in range(2):
                        cps = psx.tile([P, D + 1], F32, tag="cx")
                        for yt in range(6):
                            nc.tensor.matmul(
                                cps,
                                lhsT=pts[yt][:, xt_i * P:(xt_i + 1) * P],
                                rhs=vsb[c * 2 + yt][:, h * (D + 1):(h + 1) * (D + 1)],
                                start=(yt == 0), stop=(yt == 5),
                            )
                        rec = recp.tile([P, 1], F32, tag="rec")
                        nc.vector.reciprocal(rec, cps[:, D:D + 1])
                        nc.vector.tensor_scalar_mul(
                            osb[xt_i][:, h * D:(h + 1) * D], cps[:, :D], rec)
                for xt_i in range(2):
                    nc.sync.dma_start(
                        out=out_d[c * W + xt_i * P: c * W + (xt_i + 1) * P, :],
                        in_=osb[xt_i])
    nc.finalize()
    return nc


def _prep_inputs(hidden_states, attention_mask, Wq, bq, Wk, bk, Wv, bv, np_mm):
    scale = 1.0 / math.sqrt(D)
    Wcat = np.zeros((KIN, WCOLS), np.float32)
    Wcat[:E, :E] = Wq.T * scale
    Wcat[E, :E] = bq * scale
    Wcat[:E, E:2 * E] = Wk.T
    Wcat[E, E:2 * E] = bk
    for h in range(H):
        c0 = 2 * E + h * (D + 1)
        Wcat[:E, c0:c0 + D] = Wv.T[:, h * D:(h + 1) * D]
        Wcat[E, c0:c0 + D] = bv[h * D:(h + 1) * D]
        Wcat[E, c0 + D] = 1.0
    Wcat = Wcat.astype(np_mm)

    band = (np.arange(3 * W)[:, None] - np.arange(W)[None, :])
    bandT = ((band >= 0) & (band <= 2 * W)).astype(np_mm)

    fm_full = np.where(attention_mask[:, 0, 0, :] != 0.0,
                       np.float32(-10000.0), np.float32(0.0))

    in_maps = []
    for core in range(NCORES):
        b, g = core // CHUNKS_PER_CORE, core % CHUNKS_PER_CORE
        qlo = g * SEQ_Q
        lo, hi = qlo - W, qlo + SEQ_Q + W
        clo, chi = max(lo, 0), min(hi, S)
        xT = np.zeros((KIN, SEQ_HALO), np.float32)
        xT[:E, clo - lo:chi - lo] = hidden_states[b, clo:chi, :].T
        xT[E, :] = 1.0
        fm = np.full((SEQ_HALO,), NEG, np.float32)
        fm[clo - lo:chi - lo] = fm_full[b, clo:chi]
        in_maps.append({
            "xT": np.ascontiguousarray(xT.astype(np_mm)),
            "Wcat": Wcat,
            "fm": np.ascontiguousarray(fm.reshape(SEQ_HALO // P, P).T),
            "bandT": bandT,
        })
    return in_maps


def _get_program(dt_mm, reps=1):
    key = (dt_mm, reps)
    if key not in _cached:
        _cached[key] = _build_program(dt_mm, reps)
    return _cached[key]


def run(hidden_states, attention_mask, Wq, bq, Wk, bk, Wv, bv,
        dt_mm=BF16, np_mm=ml_dtypes.bfloat16, **run_kwargs):
    nc = _get_program(dt_mm)
    in_maps = _prep_inputs(
        np.asarray(hidden_states, np.float32),
        np.asarray(attention_mask, np.float32),
        np.asarray(Wq, np.float32), np.asarray(bq, np.float32),
        np.asarray(Wk, np.float32), np.asarray(bk, np.float32),
        np.asarray(Wv, np.float32), np.asarray(bv, np.float32),
        np_mm)
    res = run_bass_kernel_spmd(nc, in_maps, list(range(NCORES)), **run_kwargs)
    out = np.empty((B, S, E), np.float32)
    for core in range(NCORES):
        b, g = core // CHUNKS_PER_CORE, core % CHUNKS_PER_CORE
        out[b, g * SEQ_Q:(g + 1) * SEQ_Q, :] = res.results[core]["out"]
    return out, res


def kernel(**inputs):
    out, _ = run(**inputs)
    return out


def bench(inputs, iters=20, dt_mm=BF16, np_mm=None, reps=1):
    """Steady-state device timing: jitted shard_map over 8 cores, inputs
    device-resident, no donation (kernel writes every output element)."""
    import time
    import jax
    import ml_dtypes
    from jax.experimental.shard_map import shard_map
    from jax.sharding import Mesh, PartitionSpec
    from concourse import bass2jax, mybir as mb

    if np_mm is None:
        np_mm = ml_dtypes.bfloat16 if dt_mm == BF16 else np.float32
    nc = _get_program(dt_mm, reps)
    in_maps = _prep_inputs(
        np.asarray(inputs["hidden_states"], np.float32),
        np.asarray(inputs["attention_mask"], np.float32),
        np.asarray(inputs["Wq"], np.float32), np.asarray(inputs["bq"], np.float32),
        np.asarray(inputs["Wk"], np.float32), np.asarray(inputs["bk"], np.float32),
        np.asarray(inputs["Wv"], np.float32), np.asarray(inputs["bv"], np.float32),
        np_mm)
    bass2jax.install_neuronx_cc_hook()

    pid_name = nc.partition_id_tensor.name if nc.partition_id_tensor else None
    in_names, out_names, out_avals = [], [], []
    for alloc in nc.m.functions[0].allocations:
        if not isinstance(alloc, mb.MemoryLocationSet):
            continue
        name = alloc.memorylocations[0].name
        if alloc.kind == "ExternalInput":
            if name != pid_name:
                in_names.append(name)
        elif alloc.kind == "ExternalOutput":
            out_names.append(name)
            out_avals.append(jax.core.ShapedArray(
                tuple(alloc.tensor_shape), mb.dt.np(alloc.dtype)))
    bind_names = list(in_names) + ([pid_name] if pid_name else [])

    def _body(*args):
        operands = list(args)
        if pid_name:
            operands.append(bass2jax.partition_id_tensor())
        return tuple(bass2jax._bass_exec_p.bind(
            *operands, out_avals=tuple(out_avals), in_names=tuple(bind_names),
            out_names=tuple(out_names), lowering_input_output_aliases=(),
            sim_require_finite=True, sim_require_nnan=True, nc=nc))

    devices = jax.devices()[:NCORES]
    mesh = Mesh(np.asarray(devices), ("core",))
    fn = jax.jit(shard_map(
        _body, mesh=mesh,
        in_specs=(PartitionSpec("core"),) * len(in_names),
        out_specs=(PartitionSpec("core"),) * len(out_names),
        check_rep=False), keep_unused=True)
    from jax.sharding import NamedSharding
    shard = NamedSharding(mesh, PartitionSpec("core"))
    concat_in = [
        jax.device_put(
            np.concatenate([in_maps[c][n] for c in range(NCORES)], 0), shard)
        for n in in_names]
    r = fn(*concat_in)
    jax.block_until_ready(r)
    times = []
    for _ in range(iters):
        t0 = time.perf_counter()
        r = fn(*concat_in)
        jax.block_until_ready(r)
        times.append(time.perf_counter() - t0)
    out = np.asarray(r[out_names.index("out")]).reshape(NCORES, SEQ_Q, E)
    full = np.empty((B, S, E), np.float32)
    for core in range(NCORES):
        b, g = core // CHUNKS_PER_CORE, core % CHUNKS_PER_CORE
        full[b, g * SEQ_Q:(g + 1) * SEQ_Q, :] = out[core]
    return full, times
